# revision 5
# baseline (speedup 1.0000x reference)
"""Trainium2 Bass kernel: dense transformer block (B=2, S=2048, D=1024, H=16, DFF=4096).

Strategy: sequence-parallel across 8 NeuronCores (2 batches x 4 cores). Each core
owns 4 query-chunks of 128 tokens, interleaved {j, 7-j, 8+j, 15-j} so causal
attention work is balanced; per-core causal depth is padded to fixed slot budgets
(16, 12, 8, 4) with host-supplied 0/1 masks making the padding exact. K^T and V
are shared within each batch group via two AllGathers. All tensors stay
feature-major (X^T = [d, tokens]) so projections/FFN need no transposes and all
biases are per-partition. Wo@Wp is fused on the host; 1/sqrt(64) is folded into
Wq (exact power of two). All matmuls run as float32r (~1.5e-4 rel err).
"""
import numpy as np

B, S, D, H, W, DFF = 2, 2048, 1024, 16, 64, 4096
N_CORES = 8
TOK = 512            # tokens per core
NKB = 16             # key blocks (of 128 tokens) per batch
HALVES = ((0, 8), (8, 16))   # kblock halves for the attention pass

_CACHE = {}


def _chunk_rank_slot(c):
    """Global 128-token chunk c (0..15) -> (group-rank, slot). Rank j owns
    chunks {j, 7-j, 8+j, 15-j}, stored in slot order sorted by causal depth
    descending: slots = [15-j, 8+j, 7-j, j]."""
    if c < 4:
        return c, 3
    if c < 8:
        return 7 - c, 2
    if c < 12:
        return c - 8, 1
    return 15 - c, 0


def _rank_chunks(j):
    """Slot s -> global chunk for group-rank j."""
    return [15 - j, 8 + j, 7 - j, j]


def _width(t):
    """Prefix width of valid q columns for kblock t (slot budgets 16/12/8/4)."""
    return 512 - 128 * (t // 4)


def _build_module(debug=False):
    import concourse.bacc as bacc
    import concourse.tile as tile
    from concourse import mybir

    F32 = mybir.dt.float32
    F32R = mybir.dt.float32r
    AF = mybir.ActivationFunctionType
    Alu = mybir.AluOpType

    nc = bacc.Bacc("TRN2", target_bir_lowering=False, debug=False,
                   num_devices=N_CORES)

    # ---- per-core inputs ----
    xT_d = nc.dram_tensor("xT", [D, TOK], F32R, kind="ExternalInput").ap()
    mask_d = nc.dram_tensor("mask", [NKB, 128, 128], F32R,
                            kind="ExternalInput").ap()
    # ---- shared inputs (same data on every core) ----
    wq_d = nc.dram_tensor("wq", [8, 128, D], F32R, kind="ExternalInput").ap()
    wk_d = nc.dram_tensor("wk", [8, 128, D], F32R, kind="ExternalInput").ap()
    wv_d = nc.dram_tensor("wv", [8, 128, D], F32R, kind="ExternalInput").ap()
    wop_d = nc.dram_tensor("wop", [8, 128, D], F32R, kind="ExternalInput").ap()
    w1_d = nc.dram_tensor("w1", [32, 8, 128, 128], F32R,
                          kind="ExternalInput").ap()
    w2_d = nc.dram_tensor("w2", [32, 8, 128, 128], F32R,
                          kind="ExternalInput").ap()
    bq_d = nc.dram_tensor("bq", [D], F32, kind="ExternalInput").ap()
    bk_d = nc.dram_tensor("bk", [D], F32, kind="ExternalInput").ap()
    bop_d = nc.dram_tensor("bop", [D], F32, kind="ExternalInput").ap()
    b1_d = nc.dram_tensor("b1", [DFF], F32, kind="ExternalInput").ap()
    b2_d = nc.dram_tensor("b2", [D], F32, kind="ExternalInput").ap()
    sel_d = nc.dram_tensor("sel", [16, D], F32R, kind="ExternalInput").ap()
    ones_d = nc.dram_tensor("ones", [128, 16], F32R, kind="ExternalInput").ap()

    out_d = nc.dram_tensor("outT", [D, TOK], F32, kind="ExternalOutput").ap()
    if debug:
        dbg_qT = nc.dram_tensor("dbg_qT", [D, TOK], F32, kind="ExternalOutput").ap()
        dbg_ktg = nc.dram_tensor("dbg_ktg", [4 * D, TOK], F32, kind="ExternalOutput").ap()
        dbg_vg = nc.dram_tensor("dbg_vg", [4 * TOK, D], F32, kind="ExternalOutput").ap()
        dbg_attnU = nc.dram_tensor("dbg_attnU", [65, 16, TOK], F32, kind="ExternalOutput").ap()
        dbg_sums = nc.dram_tensor("dbg_sums", [16, TOK], F32, kind="ExternalOutput").ap()
        dbg_attnT = nc.dram_tensor("dbg_attnT", [D, TOK], F32, kind="ExternalOutput").ap()
        dbg_hresT = nc.dram_tensor("dbg_hresT", [D, TOK], F32, kind="ExternalOutput").ap()
        dbg_gelu = nc.dram_tensor("dbg_gelu", [DFF, TOK], F32, kind="ExternalOutput").ap()

    groups = [[0, 1, 2, 3], [4, 5, 6, 7]]

    with tile.TileContext(nc) as tc:
        with (
            tc.tile_pool(name="const", bufs=1) as constp,
            tc.tile_pool(name="mid", bufs=1) as midp,
            tc.tile_pool(name="osb", bufs=3) as osbp,
            tc.tile_pool(name="ps_main", bufs=3, space="PSUM") as psm,
            tc.tile_pool(name="ps_pv", bufs=2, space="PSUM") as pspv,
            tc.tile_pool(name="ps_bc", bufs=2, space="PSUM") as psbc,
            tc.tile_pool(name="dram", bufs=1, space="DRAM") as dramp,
        ):
            # ---------- persistent loads ----------
            xT = constp.tile([128, 8, TOK], F32R, name="xT")
            nc.sync.dma_start(xT[:], xT_d.rearrange("(k p) t -> p k t", p=128))
            mask_t = constp.tile([128, NKB, 128], F32R, name="mask_t")
            nc.sync.dma_start(mask_t[:], mask_d.rearrange("t p q -> p t q"))
            sel_t = constp.tile([16, D], F32R, name="sel_t")
            nc.sync.dma_start(sel_t[:], sel_d)
            bq_t = constp.tile([128, 8], F32, name="bq_t")
            nc.sync.dma_start(bq_t[:], bq_d.rearrange("(m p) -> p m", p=128))
            bk_t = constp.tile([128, 8], F32, name="bk_t")
            nc.sync.dma_start(bk_t[:], bk_d.rearrange("(m p) -> p m", p=128))
            bop_t = constp.tile([128, 8], F32, name="bop_t")
            nc.sync.dma_start(bop_t[:], bop_d.rearrange("(m p) -> p m", p=128))
            b1_t = constp.tile([128, 32], F32, name="b1_t")
            nc.sync.dma_start(b1_t[:], b1_d.rearrange("(m p) -> p m", p=128))
            b2_t = constp.tile([128, 8], F32, name="b2_t")
            nc.sync.dma_start(b2_t[:], b2_d.rearrange("(m p) -> p m", p=128))

            # cross-phase tiles: attn output (f32r) and attn+proj residual
            attnT = midp.tile([128, 8, TOK], F32R, name="attnT")
            hresT = midp.tile([128, 8, TOK], F32R, name="hresT")

            # DRAM bounce + gather buffers for the collectives
            kt_dram = dramp.tile([D, TOK], F32R, name="kt_dram")
            ktg_dram = dramp.tile([4 * D, TOK], F32R, name="ktg_dram")
            v_dram = dramp.tile([TOK, D], F32R, name="v_dram")
            vg_dram = dramp.tile([4 * TOK, D], F32R, name="vg_dram")

            with tc.tile_pool(name="qkt", bufs=1) as qktp:
                qT = qktp.tile([128, 8, TOK], F32R, name="qT")

                # ---------- QKV projections ----------
                with (
                    tc.tile_pool(name="wproj", bufs=2) as wprojp,
                    tc.tile_pool(name="workA", bufs=3) as workA,
                ):
                    # K^T -> AllGather
                    wk_t = wprojp.tile([128, 8, D], F32R, name="wk_t",
                                       tag="wproj")
                    nc.sync.dma_start(wk_t[:], wk_d.rearrange("k p d -> p k d"))
                    for m in range(8):
                        pp = psm.tile([128, TOK], F32, name="pp_k",
                                      tag="psmain")
                        for k in range(8):
                            nc.tensor.matmul(
                                pp[:], wk_t[:, k, m * 128:(m + 1) * 128],
                                xT[:, k, :], start=(k == 0), stop=(k == 7))
                        kt_sb = workA.tile([128, TOK], F32R, name="kt_sb",
                                           tag="work")
                        nc.scalar.activation(kt_sb[:], pp[:], AF.Identity,
                                             bias=bk_t[:, m:m + 1])
                        nc.sync.dma_start(
                            kt_dram[m * 128:(m + 1) * 128, :], kt_sb[:])
                    nc.gpsimd.collective_compute(
                        "AllGather", Alu.bypass, replica_groups=groups,
                        ins=[kt_dram.opt()], outs=[ktg_dram.opt()])

                    # V -> AllGather
                    wv_t = wprojp.tile([128, 8, D], F32R, name="wv_t",
                                       tag="wproj")
                    nc.sync.dma_start(wv_t[:], wv_d.rearrange("k p d -> p k d"))
                    for tc4 in range(4):
                        for half in range(2):
                            pp = psm.tile([128, 512], F32, name="pp_v",
                                          tag="psmain")
                            for k in range(8):
                                nc.tensor.matmul(
                                    pp[:], xT[:, k, tc4 * 128:(tc4 + 1) * 128],
                                    wv_t[:, k, half * 512:(half + 1) * 512],
                                    start=(k == 0), stop=(k == 7))
                            v_sb = workA.tile([128, 512], F32R, name="v_sb",
                                              tag="work")
                            nc.vector.tensor_copy(v_sb[:], pp[:])
                            nc.sync.dma_start(
                                v_dram[tc4 * 128:(tc4 + 1) * 128,
                                       half * 512:(half + 1) * 512], v_sb[:])
                    nc.gpsimd.collective_compute(
                        "AllGather", Alu.bypass, replica_groups=groups,
                        ins=[v_dram.opt()], outs=[vg_dram.opt()])

                    # Q^T (host folded 1/8 into wq/bq)
                    wq_t = wprojp.tile([128, 8, D], F32R, name="wq_t",
                                       tag="wproj")
                    nc.sync.dma_start(wq_t[:], wq_d.rearrange("k p d -> p k d"))
                    for m in range(8):
                        pp = psm.tile([128, TOK], F32, name="pp_q",
                                      tag="psmain")
                        for k in range(8):
                            nc.tensor.matmul(
                                pp[:], wq_t[:, k, m * 128:(m + 1) * 128],
                                xT[:, k, :], start=(k == 0), stop=(k == 7))
                        nc.scalar.activation(qT[:, m, :], pp[:], AF.Identity,
                                             bias=bq_t[:, m:m + 1])

                if debug:
                    for m in range(8):
                        nc.sync.dma_start(dbg_qT[m * 128:(m + 1) * 128, :], qT[:, m, :].bitcast(F32))
                    nc.sync.dma_start(dbg_ktg[:], ktg_dram[:].bitcast(F32))
                    nc.sync.dma_start(dbg_vg[:], vg_dram[:].bitcast(F32))

                # ---------- attention ----------
                with (
                    tc.tile_pool(name="kv", bufs=1) as kvp,
                    tc.tile_pool(name="workB", bufs=3) as workB,
                    tc.tile_pool(name="attnu", bufs=1) as attnup,
                ):
                    attnU = attnup.tile([65, 16, TOK], F32, name="attnU")
                    for hi, (t0, t1) in enumerate(HALVES):
                        kth = {}
                        vth = {}
                        for t in range(t0, t1):
                            r, s = _chunk_rank_slot(t)
                            kt_t = kvp.tile([128, 8, 128], F32R,
                                            name=f"kt_{t}", tag=f"kth{t % 8}")
                            nc.sync.dma_start(
                                kt_t[:],
                                ktg_dram[r * D:(r + 1) * D,
                                         s * 128:(s + 1) * 128]
                                .rearrange("(k p) q -> p k q", p=128))
                            kth[t] = kt_t
                            # V rows for chunk t + per-head ones column:
                            # [128 tok, 8 dchunk, 130]; per dchunk block:
                            # [V_h_even(64) | 1 | V_h_odd(64) | 1]
                            v_t = kvp.tile([128, 8, 130], F32R,
                                           name=f"v_{t}", tag=f"vth{t % 8}")
                            nc.sync.dma_start(
                                v_t[:]
                                .rearrange("p k (hh w) -> p k hh w", hh=2)
                                [:, :, :, 0:64],
                                vg_dram[r * TOK + s * 128:
                                        r * TOK + (s + 1) * 128, :]
                                .rearrange("p (k hh w) -> p k hh w",
                                           k=8, hh=2))
                            nc.sync.dma_start(
                                v_t[:]
                                .rearrange("p k (hh w) -> p k hh w", hh=2)
                                [:, :, :, 64:65],
                                ones_d.rearrange("p (k hh) -> p k hh", k=8))
                            vth[t] = v_t
                        for h in range(16):
                            m2, h2 = h // 2, h % 2
                            pv = pspv.tile([65, TOK], F32, name="pv", tag="pv")
                            for t in range(t0, t1):
                                wt = _width(t)
                                st = psm.tile([128, 512], F32, name="st",
                                              tag="psmain")
                                nc.tensor.matmul(
                                    st[:, 0:wt],
                                    kth[t][h2 * 64:(h2 + 1) * 64, m2, :],
                                    qT[h2 * 64:(h2 + 1) * 64, m2, 0:wt],
                                    start=True, stop=True)
                                pr = workB.tile([128, 512], F32R, name="pr",
                                                tag="pr")
                                nc.scalar.activation(pr[:, 0:wt], st[:, 0:wt],
                                                     AF.Exp)
                                nc.vector.tensor_mul(pr[:, wt - 128:wt],
                                                     pr[:, wt - 128:wt],
                                                     mask_t[:, t, :])
                                nc.tensor.matmul(
                                    pv[:, 0:wt],
                                    vth[t][:, m2, h2 * 65:h2 * 65 + 65],
                                    pr[:, 0:wt],
                                    start=(t == t0), stop=(t == t1 - 1))
                            # pv only has valid data in the widest prefix of
                            # this half; beyond that is unwritten PSUM
                            wmax = _width(t0)
                            if hi == 0:
                                nc.vector.tensor_copy(attnU[:, h, 0:wmax],
                                                      pv[:, 0:wmax])
                            else:
                                nc.vector.tensor_add(attnU[:, h, 0:wmax],
                                                     attnU[:, h, 0:wmax],
                                                     pv[:, 0:wmax])

                    # ---------- softmax normalization ----------
                    if debug:
                        nc.sync.dma_start(dbg_attnU[:], attnU[:])

                    # partition-scatter the sums row [1,16,TOK] -> [16,TOK]
                    # via SBUF->SBUF DMA (engines can't write partition h)
                    sums = attnup.tile([16, TOK], F32, name="sums")
                    nc.sync.dma_start(sums[:], attnU[64:65, :, :])
                    if debug:
                        nc.sync.dma_start(dbg_sums[:], sums[:])
                    recip = attnup.tile([16, TOK], F32, name="recip")
                    nc.vector.reciprocal(recip[:], sums[:])
                    recipr = attnup.tile([16, TOK], F32R, name="recipr")
                    nc.vector.tensor_copy(recipr[:], recip[:])
                    for m in range(8):
                        bc = psbc.tile([128, TOK], F32, name="bc", tag="bc")
                        nc.tensor.matmul(bc[:],
                                         sel_t[:, m * 128:(m + 1) * 128],
                                         recipr[:], start=True, stop=True)
                        for h2 in range(2):
                            h = 2 * m + h2
                            nc.vector.tensor_mul(
                                attnT[h2 * 64:(h2 + 1) * 64, m, :],
                                attnU[0:64, h, :],
                                bc[h2 * 64:(h2 + 1) * 64, :])

            if debug:
                for m in range(8):
                    nc.sync.dma_start(dbg_attnT[m * 128:(m + 1) * 128, :], attnT[:, m, :].bitcast(F32))

            # ---------- output projection (Wo@Wp fused) + residual ----------
            with tc.tile_pool(name="wop", bufs=1) as wopp:
                wop_t = wopp.tile([128, 8, D], F32R, name="wop_t")
                nc.sync.dma_start(wop_t[:], wop_d.rearrange("k p d -> p k d"))
                for m in range(8):
                    pp = psm.tile([128, TOK], F32, name="pp_o", tag="psmain")
                    for k in range(8):
                        nc.tensor.matmul(
                            pp[:], wop_t[:, k, m * 128:(m + 1) * 128],
                            attnT[:, k, :], start=(k == 0), stop=(k == 7))
                    nc.vector.scalar_tensor_tensor(
                        hresT[:, m, :], pp[:], bop_t[:, m:m + 1], xT[:, m, :],
                        op0=Alu.add, op1=Alu.add)

            if debug:
                for m in range(8):
                    nc.sync.dma_start(dbg_hresT[m * 128:(m + 1) * 128, :], hresT[:, m, :].bitcast(F32))

            # ---------- FFN ----------
            with (
                tc.tile_pool(name="gelu", bufs=1) as gelup,
                tc.tile_pool(name="wffn", bufs=3) as wffnp,
            ):
                geluT = gelup.tile([128, 32, TOK], F32R, name="geluT")
                for nf in range(32):
                    w1_t = wffnp.tile([128, 8, 128], F32R, name="w1_t",
                                      tag="w1")
                    nc.sync.dma_start(w1_t[:],
                                      w1_d[nf].rearrange("k p q -> p k q"))
                    pp = psm.tile([128, TOK], F32, name="pp_f1", tag="psmain")
                    for k in range(8):
                        nc.tensor.matmul(pp[:], w1_t[:, k, :], hresT[:, k, :],
                                         start=(k == 0), stop=(k == 7))
                    nc.scalar.activation(geluT[:, nf, :], pp[:], AF.Gelu,
                                         bias=b1_t[:, nf:nf + 1])
                if debug:
                    for nf in range(32):
                        nc.sync.dma_start(dbg_gelu[nf * 128:(nf + 1) * 128, :], geluT[:, nf, :].bitcast(F32))
                for m in range(8):
                    w2_t = wffnp.tile([128, 32, 128], F32R, name="w2_t",
                                      tag="w2", bufs=2)
                    nc.sync.dma_start(w2_t[:],
                                      w2_d[:, m].rearrange("f p q -> p f q"))
                    pp = psm.tile([128, TOK], F32, name="pp_f2", tag="psmain")
                    for kf in range(32):
                        nc.tensor.matmul(pp[:], w2_t[:, kf, :],
                                         geluT[:, kf, :], start=(kf == 0),
                                         stop=(kf == 31))
                    out_sb = osbp.tile([128, TOK], F32, name="out_sb",
                                       tag="osb")
                    nc.vector.scalar_tensor_tensor(
                        out_sb[:], pp[:], b2_t[:, m:m + 1], hresT[:, m, :],
                        op0=Alu.add, op1=Alu.add)
                    nc.sync.dma_start(out_d[m * 128:(m + 1) * 128, :],
                                      out_sb[:])

    nc.compile()
    return nc


def _get_module():
    if "nc" not in _CACHE:
        _CACHE["nc"] = _build_module()
    return _CACHE["nc"]


def _prep_shared(Wq, bq, Wk, bk, Wv, bv, Wo, bo, Wp, bp, W1, b1, W2, b2):
    """Host-side weight preprocessing (fp32 in, blocked fp32 out)."""
    Wq_s = (Wq.astype(np.float64) * 0.125).astype(np.float32)
    bq_s = (bq.astype(np.float64) * 0.125).astype(np.float32)
    Wop = (Wo.astype(np.float64) @ Wp.astype(np.float64)).astype(np.float32)
    bop = (bv.astype(np.float64) @ Wo.astype(np.float64) @ Wp.astype(np.float64)
           + bo.astype(np.float64) @ Wp.astype(np.float64)
           + bp.astype(np.float64)).astype(np.float32)
    return {
        "wq": np.ascontiguousarray(Wq_s.reshape(8, 128, D)),
        "wk": np.ascontiguousarray(Wk.reshape(8, 128, D)),
        "wv": np.ascontiguousarray(Wv.reshape(8, 128, D)),
        "wop": np.ascontiguousarray(Wop.reshape(8, 128, D)),
        "w1": np.ascontiguousarray(
            W1.reshape(8, 128, 32, 128).transpose(2, 0, 1, 3)),
        "w2": np.ascontiguousarray(
            W2.reshape(32, 128, 8, 128).transpose(0, 2, 1, 3)),
        "bq": bq_s, "bk": bk, "bop": bop, "b1": b1, "b2": b2,
        "sel": np.ascontiguousarray(
            (np.arange(D)[None, :] // 64 == np.arange(16)[:, None])
            .astype(np.float32)),
        "ones": np.ones((128, 16), np.float32),
    }


def _prep_core(x, core):
    """Per-core inputs: xT (feature-major, slot order) and causal mask."""
    b, j = core // 4, core % 4
    chunks = _rank_chunks(j)
    xc = np.concatenate(
        [x[b, c * 128:(c + 1) * 128, :] for c in chunks], axis=0)  # [512, D]
    xT = np.ascontiguousarray(xc.T)  # [D, 512]
    mask = np.zeros((NKB, 128, 128), np.float32)
    ki = np.arange(128)[:, None]
    qi = np.arange(128)[None, :]
    for t in range(NKB):
        s = 3 - t // 4
        c = chunks[s]
        mask[t] = ((c * 128 + qi) >= (t * 128 + ki)).astype(np.float32)
    return {"xT": xT, "mask": mask}


def kernel(x, Wq, bq, Wk, bk, Wv, bv, Wo, bo, Wp, bp, W1, b1, W2, b2):
    from concourse.bass_utils import run_bass_kernel_spmd

    x = np.asarray(x, np.float32)
    shared = _prep_shared(np.asarray(Wq), np.asarray(bq), np.asarray(Wk),
                          np.asarray(bk), np.asarray(Wv), np.asarray(bv),
                          np.asarray(Wo), np.asarray(bo), np.asarray(Wp),
                          np.asarray(bp), np.asarray(W1), np.asarray(b1),
                          np.asarray(W2), np.asarray(b2))
    in_maps = []
    for c in range(N_CORES):
        m = dict(shared)
        m.update(_prep_core(x, c))
        in_maps.append(m)

    nc = _get_module()
    res = run_bass_kernel_spmd(nc, in_maps, core_ids=list(range(N_CORES)))
    _CACHE["last_results"] = res

    out = np.empty((B, S, D), np.float32)
    for c in range(N_CORES):
        b, j = c // 4, c % 4
        chunks = _rank_chunks(j)
        outT = res.results[c]["outT"]  # [D, 512]
        for s, ch in enumerate(chunks):
            out[b, ch * 128:(ch + 1) * 128, :] = \
                outT[:, s * 128:(s + 1) * 128].T
    return out
